# revision 1
# baseline (speedup 1.0000x reference)
"""Trainium2 Bass kernel for NeuralBlochRK4.

Reference computation: RK4 integration (255 steps) of dy/dt = MLP([y,u(t),p,t])
with MLP 13 -> 128(tanh) -> 128(tanh) -> 3, batch 16384, output = full
trajectory (B, 256, 3).

Strategy (pure data-parallel over batch, 8 cores x 2048 rows):
  * All elementwise adds are folded into PSUM matmul accumulation or the
    ACT engine's free affine (out = tanh(in + bias)).
  * Per stage s of RK4, z1 = Wc_s^T @ x (K=17 matmul over packed input tile
    [y(3); ones(1); p(5); u_n(4); u_{n+1}(4)]) + alpha_s*(W1_y @ W3) @ h2_{s-1}
    (K=128 matmul) accumulated in PSUM; tanh via ACT with per-step bias
    w_t * t_n.  z2 = W2 @ h1; tanh with bias b2.
  * y_{n+1} accumulated in a separate PSUM group: I-matmul (adds y_n + h*b3
    via a ones-row) + four gamma_s*W3 @ h2_s matmuls.
  * Batch is split into 2 interleaved "threads" of 1024 per core so ACT/PE
    pipeline across threads; ACT (tanh @ 1 elem/cycle/lane) is the binding
    engine.
  * u is pre-transposed on host to (T*4, B_core) so the per-step (8, W) DMA
    slices are contiguous.
"""

import numpy as np
from contextlib import ExitStack

import concourse.bass as bass
import concourse.tile as tile
from concourse import bacc, mybir
from concourse.bass_utils import run_bass_kernel_spmd

F32 = mybir.dt.float32
TANH = mybir.ActivationFunctionType.Tanh

B_FULL, T_FULL, HID = 16384, 256, 128
N_CORES = 8


# ----------------------------------------------------------------------------
# host-side constant preparation
# ----------------------------------------------------------------------------

def prepare_consts(W1, b1, W2, b2, W3, b3, t):
    f32 = np.float32
    W1 = np.asarray(W1, f32); W2 = np.asarray(W2, f32); W3 = np.asarray(W3, f32)
    b1 = np.asarray(b1, f32); b2 = np.asarray(b2, f32); b3 = np.asarray(b3, f32)
    t = np.asarray(t, f32)
    h = f32(t[1] - t[0])

    A = W1[:, 0:3]
    U = W1[:, 3:7]
    P = W1[:, 7:12]
    w_t = W1[:, 12]
    C = (A @ W3).astype(f32)
    Ab3 = (A @ b3).astype(f32)

    stages = [
        (f32(0.0), f32(0.0), f32(1.0), f32(0.0)),
        (f32(h / 2), f32(h / 2), f32(0.5), f32(0.5)),
        (f32(h / 2), f32(h / 2), f32(0.5), f32(0.5)),
        (f32(h), f32(h), f32(0.0), f32(1.0)),
    ]
    Wc = []
    for (o, al, cn, ce) in stages:
        kxm = np.zeros((17, 128), f32)
        kxm[0:3, :] = A.T
        kxm[3, :] = b1 + w_t * o + al * Ab3
        kxm[4:9, :] = P.T
        kxm[9:13, :] = cn * U.T
        kxm[13:17, :] = ce * U.T
        Wc.append(np.ascontiguousarray(kxm))

    consts = {
        "Wc1": Wc[0], "Wc23": Wc[1], "Wc4": Wc[3],
        "Ch": np.ascontiguousarray((f32(h / 2) * C.T).astype(f32)),
        "Cf": np.ascontiguousarray((f32(h) * C.T).astype(f32)),
        "W2T": np.ascontiguousarray(W2.T.astype(f32)),
        "W36": np.ascontiguousarray((f32(h / 6) * W3.T).astype(f32)),
        "W33": np.ascontiguousarray((f32(h / 3) * W3.T).astype(f32)),
        "wtt": np.ascontiguousarray(np.outer(w_t, t).astype(f32)),
        "b2": np.ascontiguousarray(b2.reshape(128, 1)),
    }
    I4 = np.zeros((4, 3), f32)
    I4[0:3, 0:3] = np.eye(3, dtype=f32)
    I4[3, :] = h * b3
    consts["I4"] = I4
    return consts


# ----------------------------------------------------------------------------
# device program
# ----------------------------------------------------------------------------

def build_tile_body(tc, aps, B_core, T, NTH):
    """Emit the full unrolled RK4 program into TileContext `tc`.

    aps: dict name -> bass.AP for all DRAM tensors.
    """
    nc = tc.nc
    W = B_core // NTH          # per-thread batch width
    CH = min(512, W)           # matmul free-dim chunk (one PSUM bank)
    NCH = W // CH
    assert W % CH == 0 and B_core % NTH == 0

    with ExitStack() as ctx:
        wpool = ctx.enter_context(tc.tile_pool(name="wts", bufs=1))
        xpool = ctx.enter_context(tc.tile_pool(name="x", bufs=1))
        h1pool = ctx.enter_context(tc.tile_pool(name="h1", bufs=2))
        h2pool = ctx.enter_context(tc.tile_pool(name="h2", bufs=3))
        zpool = ctx.enter_context(
            tc.tile_pool(name="z", bufs=2, space=bass.MemorySpace.PSUM))
        ypool = ctx.enter_context(
            tc.tile_pool(name="yp", bufs=2, space=bass.MemorySpace.PSUM))

        # ---- load constants into SBUF
        def wtile(name, shape):
            tl = wpool.tile(list(shape), F32, tag=name)
            nc.sync.dma_start(tl[:, :], aps[name][:, :])
            return tl

        wc1 = wtile("Wc1", (17, 128))
        wc23 = wtile("Wc23", (17, 128))
        wc4 = wtile("Wc4", (17, 128))
        ch_t = wtile("Ch", (128, 128))
        cf_t = wtile("Cf", (128, 128))
        w2t = wtile("W2T", (128, 128))
        w36 = wtile("W36", (128, 3))
        w33 = wtile("W33", (128, 3))
        i4 = wtile("I4", (4, 3))
        wtt = wtile("wtt", (128, T))
        b2t = wtile("b2", (128, 1))

        wc_s = (wc1, wc23, wc23, wc4)
        cs_s = (None, ch_t, ch_t, cf_t)
        w3_s = (w36, w33, w33, w36)

        yout = aps["yout"]      # (3, T-1, B_core)
        uT = aps["uT"]          # (T*4, B_core)
        xinit = aps["xinit"]    # (17, B_core)

        # ---- persistent x tiles: [thread][parity]
        xb = []
        for th in range(NTH):
            bufs = []
            for par in range(2):
                tl = xpool.tile([17, W], F32, tag=f"xb{th}{par}")
                nc.sync.dma_start(tl[:, :], xinit[:, th * W:(th + 1) * W])
                bufs.append(tl)
            xb.append(bufs)
        # u rows for step 0 and prefetch for step 1
        for th in range(NTH):
            nc.sync.dma_start(xb[th][0][9:17, :], uT[0:8, th * W:(th + 1) * W])
            if T - 1 > 1:
                nc.sync.dma_start(xb[th][1][9:17, :], uT[4:12, th * W:(th + 1) * W])

        def chunks(ap, rows=None):
            for c in range(NCH):
                sl = slice(c * CH, (c + 1) * CH)
                yield (ap[:, sl] if rows is None else ap[rows, sl])

        # ---- time loop (fully unrolled)
        for n in range(T - 1):
            par, nxt = n % 2, (n + 1) % 2

            # prefetch u for step n+1 into the other parity buffer
            if n + 1 <= T - 2:
                r0 = 4 * (n + 1)
                for th in range(NTH):
                    nc.sync.dma_start(xb[th][nxt][9:17, :],
                                      uT[r0:r0 + 8, th * W:(th + 1) * W])

            # seed y accumulation: y_n + h*b3  (I4 ones-row trick)
            ypsum = []
            for th in range(NTH):
                yp = ypool.tile([3, W], F32, tag="yp")
                for c in range(NCH):
                    sl = slice(c * CH, (c + 1) * CH)
                    nc.tensor.matmul(yp[:, sl], i4[:, :],
                                     xb[th][par][0:4, sl],
                                     start=True, stop=False)
                ypsum.append(yp)

            h2prev = [None] * NTH
            for s in range(4):
                z1s, h1s, z2s, h2s = [], [], [], []
                for th in range(NTH):
                    z1 = zpool.tile([128, W], F32, tag="z")
                    for c in range(NCH):
                        sl = slice(c * CH, (c + 1) * CH)
                        nc.tensor.matmul(z1[:, sl], wc_s[s][:, :],
                                         xb[th][par][:, sl],
                                         start=True, stop=(s == 0))
                        if s > 0:
                            nc.tensor.matmul(z1[:, sl], cs_s[s][:, :],
                                             h2prev[th][:, sl],
                                             start=False, stop=True)
                    z1s.append(z1)
                for th in range(NTH):
                    h1 = h1pool.tile([128, W], F32, tag="h1")
                    nc.scalar.activation(h1[:, :], z1s[th][:, :], TANH,
                                         bias=wtt[:, n:n + 1])
                    h1s.append(h1)
                for th in range(NTH):
                    z2 = zpool.tile([128, W], F32, tag="z")
                    for c in range(NCH):
                        sl = slice(c * CH, (c + 1) * CH)
                        nc.tensor.matmul(z2[:, sl], w2t[:, :], h1s[th][:, sl],
                                         start=True, stop=True)
                    z2s.append(z2)
                for th in range(NTH):
                    h2 = h2pool.tile([128, W], F32, tag="h2")
                    nc.scalar.activation(h2[:, :], z2s[th][:, :], TANH,
                                         bias=b2t[:, 0:1])
                    h2s.append(h2)
                for th in range(NTH):
                    for c in range(NCH):
                        sl = slice(c * CH, (c + 1) * CH)
                        nc.tensor.matmul(ypsum[th][:, sl], w3_s[s][:, :],
                                         h2s[th][:, sl],
                                         start=False, stop=(s == 3))
                h2prev = h2s

            # y_{n+1}: PSUM -> next x buffer rows 0-2, then to DRAM
            for th in range(NTH):
                nc.vector.tensor_copy(xb[th][nxt][0:3, :], ypsum[th][:, :])
                nc.sync.dma_start(yout[:, n, th * W:(th + 1) * W],
                                  xb[th][nxt][0:3, :])


def build_program(B_core, T, NTH, debug=False, enable_asserts=False):
    nc = bacc.Bacc("TRN2", target_bir_lowering=False, debug=debug,
                   enable_asserts=enable_asserts, num_devices=1)
    shapes = {
        "xinit": (17, B_core),
        "uT": (T * 4, B_core),
        "Wc1": (17, 128), "Wc23": (17, 128), "Wc4": (17, 128),
        "Ch": (128, 128), "Cf": (128, 128), "W2T": (128, 128),
        "W36": (128, 3), "W33": (128, 3), "I4": (4, 3),
        "wtt": (128, T), "b2": (128, 1),
    }
    aps = {}
    for name, shp in shapes.items():
        aps[name] = nc.dram_tensor(name, list(shp), F32,
                                   kind="ExternalInput").ap()
    aps["yout"] = nc.dram_tensor("yout", [3, T - 1, B_core], F32,
                                 kind="ExternalOutput").ap()
    with tile.TileContext(nc) as tc:
        build_tile_body(tc, aps, B_core, T, NTH)
    nc.compile()
    return nc


def make_in_maps(y0, t, u, p, W1, b1, W2, b2, W3, b3, n_cores, B_core, T):
    f32 = np.float32
    y0 = np.asarray(y0, f32); u = np.asarray(u, f32); p = np.asarray(p, f32)
    consts = prepare_consts(W1, b1, W2, b2, W3, b3, t)
    in_maps = []
    for i in range(n_cores):
        sl = slice(i * B_core, (i + 1) * B_core)
        xinit = np.zeros((17, B_core), f32)
        xinit[0:3] = y0[sl].T
        xinit[3] = 1.0
        xinit[4:9] = p[sl].T
        uT = np.ascontiguousarray(
            u[sl].transpose(1, 2, 0).reshape(T * 4, B_core))
        m = {"xinit": xinit, "uT": uT}
        m.update(consts)
        in_maps.append(m)
    return in_maps


_PROGRAM_CACHE = {}


def _get_program(B_core, T, NTH):
    key = (B_core, T, NTH)
    if key not in _PROGRAM_CACHE:
        _PROGRAM_CACHE[key] = build_program(B_core, T, NTH)
    return _PROGRAM_CACHE[key]


def run_on_cores(inputs, n_cores=N_CORES, NTH=2, trace=False):
    """inputs: dict with full y0/t/u/p/weights. Returns (out, results_obj)."""
    y0 = np.asarray(inputs["y0"], np.float32)
    B = y0.shape[0]
    T = np.asarray(inputs["t"]).shape[0]
    B_core = B // n_cores
    nc = _get_program(B_core, T, NTH)
    in_maps = make_in_maps(
        inputs["y0"], inputs["t"], inputs["u"], inputs["p"],
        inputs["W1"], inputs["b1"], inputs["W2"], inputs["b2"],
        inputs["W3"], inputs["b3"], n_cores, B_core, T)
    res = run_bass_kernel_spmd(nc, in_maps, list(range(n_cores)), trace=trace)
    out = np.empty((B, T, 3), np.float32)
    for i in range(n_cores):
        sl = slice(i * B_core, (i + 1) * B_core)
        yo = np.asarray(res.results[i]["yout"])        # (3, T-1, B_core)
        out[sl, 1:, :] = yo.transpose(2, 1, 0)
        out[sl, 0, :] = y0[sl]
    return out, res


def kernel(y0, t, u, p, W1, b1, W2, b2, W3, b3):
    out, _ = run_on_cores(
        dict(y0=y0, t=t, u=u, p=p, W1=W1, b1=b1, W2=W2, b2=b2,
             W3=W3, b3=b3),
        n_cores=N_CORES, NTH=2, trace=False)
    return out
